# revision 7
# baseline (speedup 1.0000x reference)
"""Trainium2 Bass kernel for BilinearCategoricalNet.

  h1 = relu(relu(x1 @ m1_w1.T + m1_b1) @ m1_w2.T + m1_b2)      [B, H]
  h2 = same for x2 / m2
  o1 = einsum('bh,oph->bpo', h1, mll1_w) + mll1_b.T            [B, P, O]
  o2 = same for h2 / mll2
  logits = sum_p o1 * o2                                       [B, O]

Strategy: data-parallel over 8 cores (1024 rows each), weights replicated.
All matmuls in bf16: measured (microbench, steady chain) ~216-221ns per
128x128x512 matmul vs fp32r's ~232ns, i.e. bf16 runs at the 1 cycle/row
PE rate with less per-instr overhead, and halves SBUF/DMA footprint.
End-to-end error ~5.4e-3 max-rel vs the 2e-2 gate. fp8 DoubleRow (2x PE
rate) was evaluated and rejected: naive e4m3 quantization of the MLL
measures 5.3e-2 max-rel (fails the gate), and any hi/lo split needs >=2
full-K passes, erasing the 2x (DoubleRow K=256 takes the same time as a
bf16 K=128 instr per measured HW, not the 0.5 cycles/row CoreSim claims).
DMA note: each dma_start costs ~0.6us of serial sync-engine descriptor
time, so inputs are staged as a few large multi-chunk DMAs ordered by
first use (the PE lead-in is descriptor-count bound, not byte bound).
Activations kept feature-major [h, b] so every contraction has its
contraction dim on SBUF partitions. The MLL stage emits batch-major
[b, (o,p)] PSUM tiles so sum_p becomes a DVE free-axis segmented reduce.
MLL biases are folded into precomputed corrections:
  logits = sum_p a*c + h1 @ v1.T + h2 @ v2.T + c0
where a/c are the bias-free MLL outputs, v1[o] = sum_p mll2_b[o,p]*mll1_w[o,p],
v2[o] = sum_p mll1_b[o,p]*mll2_w[o,p], c0[o] = mll1_b[o] . mll2_b[o].
"""
import sys

sys.path.insert(0, "/opt/trn_rl_repo")

import numpy as np

B = 8192
NCORES = 8
BL = B // NCORES          # 1024 rows per core
NI = 512                  # input features
H = 1024                  # hidden
O = 128                   # num outputs
P = 64                    # pre-bilinear
OP = O * P                # 8192 flattened (o, p), o-major
KC1 = NI // 128           # 4 k-chunks, layer 1
HC = H // 128             # 8 h-chunks
BCH = BL // 512           # 2 batch chunks of 512 (MLP free dim)
BT = BL // 128            # 8 batch tiles of 128 (MLL stationary dim)
CH = OP // 512            # 16 (o,p)-chunks of 512 (= 8 o's each)

_CACHED = None


def _build(phase=3, n_chunks=CH):
    """phase: 1=MLPs only, 2=+corrections, 3=full (+MLL)."""
    import concourse.bacc as bacc
    import concourse.mybir as mybir
    from concourse.tile import TileContext

    f32 = mybir.dt.float32
    bf = mybir.dt.bfloat16
    Relu = mybir.ActivationFunctionType.Relu
    Add = mybir.AluOpType.add
    AX = mybir.AxisListType.X

    nc = bacc.Bacc("TRN2", target_bir_lowering=False, debug=False,
                   num_devices=NCORES)

    def din(name, shape, dt=bf):
        return nc.dram_tensor(name, shape, dt, kind="ExternalInput").ap()

    xT = [din("xT1", [NI, BL]), din("xT2", [NI, BL])]
    w1T = [din("w1T_1", [NI, H]), din("w1T_2", [NI, H])]
    w2T = [din("w2T_1", [H, H]), din("w2T_2", [H, H])]
    # biases packed [b1_n | b2_n] -> one DMA per net
    bias = [din("bias_1", [128, 2, HC], f32), din("bias_2", [128, 2, HC], f32)]
    mllT = [din("mllT1", [H, OP]), din("mllT2", [H, OP])]
    vT = [din("v1T", [H, O]), din("v2T", [H, O])]
    c0 = din("c0", [1, O])
    ones = din("ones", [1, 512])
    ident = din("ident", [128, 128], f32)
    out = nc.dram_tensor("out", [BL, O], f32, kind="ExternalOutput").ap()

    with TileContext(nc) as tc:
        with tc.tile_pool(name="persist", bufs=1) as pp, \
             tc.tile_pool(name="ps", bufs=1, space="PSUM") as ps:
            # final MLP outputs, feature-major [h, b] — live through MLL phase
            f_sb = [[pp.tile([128, BL], bf, name=f"f{n}_{m}") for m in range(HC)]
                    for n in range(2)]
            logits_sb = pp.tile([128, BT, O], f32, name="logits_sb")
            corr_sb = pp.tile([128, BT, O], f32, name="corr_sb")
            bias_sb = [pp.tile([128, 2, HC], f32, name=f"biassb{n}")
                       for n in range(2)]

            # ---------------- MLP phase (per net, shared slots) -------------
            # Consolidated tiles + few big DMAs: each dma_start costs a
            # ~0.6us DIRECT2D descriptor slot on the serial sync engine, so
            # the PE lead-in is gated by descriptor COUNT, not bytes.
            with tc.tile_pool(name="mlp", bufs=1) as mp:
                for n in range(2):
                    # 2 bufs per tag so net 2's DMAs prefetch during net 1
                    x_t = mp.tile([128, KC1, BL], bf, name="x_t", tag="x_t",
                                  bufs=2)
                    w1_t = mp.tile([128, KC1, H], bf, name="w1_t", tag="w1_t",
                                   bufs=2)
                    w2_t = mp.tile([128, HC, H], bf, name="w2_t", tag="w2_t",
                                   bufs=2)
                    xr = xT[n].rearrange("(kc p) b -> p kc b", p=128)
                    w1r = w1T[n].rearrange("(kc p) h -> p kc h", p=128)
                    w2r = w2T[n].rearrange("(kc p) h -> p kc h", p=128)
                    # ordered by first use (layer-1 runs bc-outer, m-inner).
                    # Bulk DMAs stay on the sync HWDGE queue: routing weights
                    # via the scalar-engine HWDGE queue was measured slower
                    # (its DIRECT2D descriptors stall the PE on gating loads).
                    # Exception: net 1's first x chunk + bias ride the idle
                    # scalar/vector queues so their ~1us DIRECT2D descriptor
                    # slots process in parallel with w1's on sync, trimming
                    # the PE lead-in (first MM gates on w1[m=0] + x[bc=0]).
                    if n == 0:
                        nc.sync.dma_start(out=w1_t[:, :, 0:128],
                                          in_=w1r[:, :, 0:128])
                        nc.scalar.dma_start(out=x_t[:, :, 0:512],
                                            in_=xr[:, :, 0:512])
                        nc.gpsimd.dma_start(out=bias_sb[n], in_=bias[n])
                        nc.sync.dma_start(out=w1_t[:, :, 128:512],
                                          in_=w1r[:, :, 128:512])
                    else:
                        nc.sync.dma_start(out=x_t[:, :, 0:512],
                                          in_=xr[:, :, 0:512])
                        nc.sync.dma_start(out=w1_t[:, :, 0:512],
                                          in_=w1r[:, :, 0:512])
                        nc.sync.dma_start(out=bias_sb[n], in_=bias[n])
                    nc.sync.dma_start(out=w1_t[:, :, 512:H],
                                      in_=w1r[:, :, 512:H])
                    nc.sync.dma_start(out=x_t[:, :, 512:BL],
                                      in_=xr[:, :, 512:BL])
                    nc.sync.dma_start(out=w2_t[:, :, 0:512],
                                      in_=w2r[:, :, 0:512])
                    nc.sync.dma_start(out=w2_t[:, :, 512:H],
                                      in_=w2r[:, :, 512:H])
                    h_t = [mp.tile([128, BL], bf, name=f"h_{m}", tag=f"h_{m}")
                           for m in range(HC)]
                    # layer 1: h[m] = relu(w1.T @ x + b1)
                    for bc in range(BCH):
                        for m in range(HC):
                            pm = ps.tile([128, 512], f32, name="pm", tag="mlp",
                                         bufs=3)
                            for kc in range(KC1):
                                nc.tensor.matmul(
                                    pm,
                                    w1_t[:, kc, m * 128:(m + 1) * 128],
                                    x_t[:, kc, bc * 512:(bc + 1) * 512],
                                    start=(kc == 0), stop=(kc == KC1 - 1))
                            nc.scalar.activation(
                                h_t[m][:, bc * 512:(bc + 1) * 512], pm, Relu,
                                bias=bias_sb[n][:, 0, m:m + 1])
                    # layer 2: f[m] = relu(w2.T @ h + b2)
                    for m in range(HC):
                        for bc in range(BCH):
                            pm = ps.tile([128, 512], f32, name="pm", tag="mlp",
                                         bufs=3)
                            for kc in range(HC):
                                nc.tensor.matmul(
                                    pm,
                                    w2_t[:, kc, m * 128:(m + 1) * 128],
                                    h_t[kc][:, bc * 512:(bc + 1) * 512],
                                    start=(kc == 0), stop=(kc == HC - 1))
                            nc.scalar.activation(
                                f_sb[n][m][:, bc * 512:(bc + 1) * 512], pm, Relu,
                                bias=bias_sb[n][:, 1, m:m + 1])

            if phase == 1:
                for m in range(HC):
                    nc.sync.dma_start(
                        out=out[m * 128:(m + 1) * 128, :],
                        in_=f_sb[0][m][:, 0:O])

            # ---------------- corrections: h1@v1.T + h2@v2.T + c0 -----------
            # computed o-major (v stationary, N=512) then PE-transposed to
            # batch-major — N=128 f-stationary matmuls are LDWEIGHTS-bound.
            v_sb = [pp.tile([128, HC, O], bf, name=f"vsb{n}") for n in range(2)]
            c0_sb = pp.tile([1, O], bf, name="c0sb")
            ones_sb = pp.tile([1, 512], bf, name="onessb")
            ident_sb = pp.tile([128, 128], f32, name="identsb")
            for n in range(2):
                nc.sync.dma_start(
                    out=v_sb[n], in_=vT[n].rearrange("(hc p) o -> p hc o", p=128))
            nc.sync.dma_start(out=c0_sb, in_=c0)
            nc.sync.dma_start(out=ones_sb, in_=ones)
            nc.sync.dma_start(out=ident_sb, in_=ident)

            # MLL weight tiles live in the persistent pool (not a scoped
            # pool) so chunk 0/1 DMAs aren't SBUF-gated on the MLP pool's
            # teardown — measured 6.2us PE stall at the MLP->MLL transition
            # when m_t could only start loading after MLP tiles died.
            def mll_dma(c):
                t = [pp.tile([128, HC, 512], bf, name=f"m{n}t", tag=f"m{n}t",
                             bufs=2) for n in range(2)]
                for n in range(2):
                    nc.sync.dma_start(
                        out=t[n],
                        in_=mllT[n].rearrange("(hc p) f -> p hc f", p=128)
                        [:, :, c * 512:(c + 1) * 512])
                return t

            m_next = mll_dma(0) if phase >= 3 else None
            corr_om = pp.tile([128, BL], f32, name="corr_om")
            for bc in range(BCH if phase >= 2 else 0):
                pc = ps.tile([128, 512], f32, name="pc", tag="mlp", bufs=3)
                first = True
                for n in range(2):
                    for hc in range(HC):
                        nc.tensor.matmul(
                            pc, v_sb[n][:, hc, :],
                            f_sb[n][hc][:, bc * 512:(bc + 1) * 512],
                            start=first, stop=False)
                        first = False
                nc.tensor.matmul(pc, c0_sb, ones_sb, start=False, stop=True)
                nc.vector.tensor_copy(corr_om[:, bc * 512:(bc + 1) * 512], pc)
            def emit_transposes():
                # corr_om (o-major) -> corr_sb (batch-major); gated on the
                # DVE copies of corr_om, so emitted after MLL chunk 0 keeps
                # the PE fed while those copies drain.
                for bt in range(BT):
                    pt = ps.tile([128, O], f32, name="pt", tag="mlp", bufs=3)
                    nc.tensor.transpose(
                        pt, corr_om[:, bt * 128:(bt + 1) * 128], ident_sb)
                    nc.vector.tensor_copy(corr_sb[:, bt, :], pt)

            if phase == 2:
                emit_transposes()
                for bt in range(BT):
                    nc.sync.dma_start(out=out[bt * 128:(bt + 1) * 128, :],
                                      in_=corr_sb[:, bt, :])

            # ---------------- MLL phase: chunk-outer, btile-inner -----------
            with tc.tile_pool(name="mll", bufs=1) as lp:
                for c in range(n_chunks if phase >= 3 else 0):
                    m_t = m_next
                    if c + 1 < n_chunks:
                        m_next = mll_dma(c + 1)
                    for bt in range(BT):
                        pr = [ps.tile([128, 512], f32, name=f"pr{n}",
                                      tag=f"pr{n}", bufs=3 if n == 0 else 2)
                              for n in range(2)]
                        # net 2 first: its PSUM->SBUF bounce copy (DVE can
                        # read only one PSUM operand) overlaps net 1's
                        # matmul group instead of serializing after it.
                        for hc in range(HC):
                            nc.tensor.matmul(
                                pr[1],
                                f_sb[1][hc][:, bt * 128:(bt + 1) * 128],
                                m_t[1][:, hc, :],
                                start=(hc == 0), stop=(hc == HC - 1))
                        o2_sb = lp.tile([128, 512], f32, name="o2_sb",
                                        tag="o2_sb", bufs=3)
                        nc.vector.tensor_copy(o2_sb, pr[1])
                        if c == 0 and bt == 0:
                            # slot the corr transposes here: the corr_om
                            # PSUM->SBUF copies they gate on drain during
                            # the pr[1] group above, so the PE never idles
                            # waiting for them.
                            emit_transposes()
                        prod = lp.tile([128, 512], f32, name="prod", tag="prod",
                                       bufs=4)
                        if c == n_chunks - 1 and bt == BT - 1:
                            # final group split in two column halves so the
                            # first half's mul+reduce pipelines under the
                            # second half's matmuls. Logits cols 0:120 are
                            # complete before this btile (chunk 15 only
                            # yields cols 120:128), so their add+store hide
                            # under the matmuls; each half then finishes
                            # with a 4-column add + store on the idle
                            # gpsimd DMA queue, shrinking the post-last-MM
                            # chain to mul+reduce+tiny-add+tiny-store.
                            o_sb = lp.tile([128, O], f32, name="o_sb",
                                           tag="o_sb", bufs=2)
                            nc.vector.tensor_add(o_sb[:, 0:120],
                                                 logits_sb[:, bt, 0:120],
                                                 corr_sb[:, bt, 0:120])
                            nc.sync.dma_start(
                                out=out[bt * 128:(bt + 1) * 128, 0:120],
                                in_=o_sb[:, 0:120])
                            # bf16 product for the final 8 columns: halves
                            # the DVE mul+reduce on the post-last-MM chain
                            # (error impact limited to 8/128 columns at
                            # ~0.05% — products are summed in fp32).
                            prod_l = lp.tile([128, 512], bf, name="prod_l",
                                             tag="prod_l", bufs=1)
                            for hf in range(2):
                                sl = slice(hf * 256, (hf + 1) * 256)
                                cl = slice(c * 8 + hf * 4, c * 8 + hf * 4 + 4)
                                for hc in range(HC):
                                    nc.tensor.matmul(
                                        pr[0][:, sl],
                                        f_sb[0][hc][:, bt * 128:(bt + 1) * 128],
                                        m_t[0][:, hc, sl],
                                        start=(hc == 0), stop=(hc == HC - 1))
                                nc.vector.tensor_mul(prod_l[:, sl],
                                                     pr[0][:, sl],
                                                     o2_sb[:, sl])
                                nc.vector.tensor_reduce(
                                    logits_sb[:, bt, cl],
                                    prod_l[:, sl].rearrange("p (o q) -> p o q",
                                                            q=P),
                                    axis=AX, op=Add)
                            nc.vector.tensor_add(o_sb[:, 120:O],
                                                 logits_sb[:, bt, 120:O],
                                                 corr_sb[:, bt, 120:O])
                            nc.gpsimd.dma_start(
                                out=out[bt * 128:(bt + 1) * 128, 120:O],
                                in_=o_sb[:, 120:O])
                        else:
                            for hc in range(HC):
                                nc.tensor.matmul(
                                    pr[0],
                                    f_sb[0][hc][:, bt * 128:(bt + 1) * 128],
                                    m_t[0][:, hc, :],
                                    start=(hc == 0), stop=(hc == HC - 1))
                            nc.vector.tensor_mul(prod, pr[0], o2_sb)
                            nc.vector.tensor_reduce(
                                logits_sb[:, bt, c * 8:(c + 1) * 8],
                                prod.rearrange("p (o q) -> p o q", q=P),
                                axis=AX, op=Add)
                        # last chunk: fused final add + store per btile, so
                        # the tail after the last matmul is one btile's DVE
                        # chain instead of 8 serialized add+DMA pairs (the
                        # very last btile is handled split-wise above).
                        if c == n_chunks - 1 and bt != BT - 1:
                            o_sb = lp.tile([128, O], f32, name="o_sb",
                                           tag="o_sb", bufs=2)
                            nc.vector.tensor_add(o_sb, logits_sb[:, bt, :],
                                                 corr_sb[:, bt, :])
                            nc.sync.dma_start(
                                out=out[bt * 128:(bt + 1) * 128, :], in_=o_sb)

    nc.compile()
    return nc


def _get_nc():
    global _CACHED
    if _CACHED is None:
        _CACHED = _build()
    return _CACHED


def _prep_shared(m1_w1, m1_b1, m1_w2, m1_b2, m2_w1, m2_b1, m2_w2, m2_b2,
                 mll1_w, mll1_b, mll2_w, mll2_b):
    """Host-side weight layouts, shared by all cores."""
    import ml_dtypes
    f = np.float32
    bf = ml_dtypes.bfloat16
    d = {}
    d["w1T_1"] = np.ascontiguousarray(m1_w1.T).astype(bf)
    d["w1T_2"] = np.ascontiguousarray(m2_w1.T).astype(bf)
    d["w2T_1"] = np.ascontiguousarray(m1_w2.T).astype(bf)
    d["w2T_2"] = np.ascontiguousarray(m2_w2.T).astype(bf)
    d["bias_1"] = np.ascontiguousarray(np.stack(
        [m1_b1.reshape(HC, 128).T, m1_b2.reshape(HC, 128).T], axis=1)).astype(f)
    d["bias_2"] = np.ascontiguousarray(np.stack(
        [m2_b1.reshape(HC, 128).T, m2_b2.reshape(HC, 128).T], axis=1)).astype(f)
    # [O, P, H] -> [H, O*P] with o-major flattened columns
    d["mllT1"] = np.ascontiguousarray(
        mll1_w.transpose(2, 0, 1).reshape(H, OP)).astype(bf)
    d["mllT2"] = np.ascontiguousarray(
        mll2_w.transpose(2, 0, 1).reshape(H, OP)).astype(bf)
    v1 = np.einsum("op,oph->oh", mll2_b.astype(np.float64),
                   mll1_w.astype(np.float64))
    v2 = np.einsum("op,oph->oh", mll1_b.astype(np.float64),
                   mll2_w.astype(np.float64))
    d["v1T"] = np.ascontiguousarray(v1.T).astype(bf)
    d["v2T"] = np.ascontiguousarray(v2.T).astype(bf)
    d["c0"] = (mll1_b.astype(np.float64) *
               mll2_b.astype(np.float64)).sum(axis=1)[None, :].astype(bf)
    d["ones"] = np.ones((1, 512), dtype=bf)
    d["ident"] = np.eye(128, dtype=f)
    return d


def kernel(x_1, x_2, m1_w1, m1_b1, m1_w2, m1_b2, m2_w1, m2_b1, m2_w2, m2_b2,
           mll1_w, mll1_b, mll2_w, mll2_b):
    from concourse.bass_utils import run_bass_kernel_spmd

    nc = _get_nc()
    shared = _prep_shared(np.asarray(m1_w1), np.asarray(m1_b1),
                          np.asarray(m1_w2), np.asarray(m1_b2),
                          np.asarray(m2_w1), np.asarray(m2_b1),
                          np.asarray(m2_w2), np.asarray(m2_b2),
                          np.asarray(mll1_w), np.asarray(mll1_b),
                          np.asarray(mll2_w), np.asarray(mll2_b))
    import ml_dtypes
    bf = ml_dtypes.bfloat16
    x_1 = np.asarray(x_1, dtype=np.float32)
    x_2 = np.asarray(x_2, dtype=np.float32)
    in_maps = []
    for c in range(NCORES):
        sl = slice(c * BL, (c + 1) * BL)
        m = dict(shared)
        m["xT1"] = np.ascontiguousarray(x_1[sl].T).astype(bf)
        m["xT2"] = np.ascontiguousarray(x_2[sl].T).astype(bf)
        in_maps.append(m)
    res = run_bass_kernel_spmd(nc, in_maps, list(range(NCORES)))
    return np.concatenate([res.results[c]["out"] for c in range(NCORES)],
                          axis=0)

